# revision 50
# baseline (speedup 1.0000x reference)
"""Multi-head dot-product attention (B=2, S=2048, F=1024, H=16, DH=64, O=1024)
as a Bass/Tile kernel on 8 Trainium2 NeuronCores.

Sharding: data-parallel over B (2) x tensor-parallel over H (4 groups of 4
heads) = 8 cores. Each core computes q/k/v projections for its 4 heads,
softmax attention, and a partial output projection; the host sums the 4
partial outputs per batch element and adds the bias.

Device layouts (per core, host-pre-packed so every DMA reads multi-KB
contiguous runs per partition instead of 1KB strided rows):
  xqT, xkvT  [128, nch, nf, CH] fp16  = xT[t*128+p, c*CH+n] at [p,c,t,n]
  wq, wk, wv [128, nf, 4*DH] fp16     (wq pre-scaled by 1/sqrt(DH))
  wo         [128, 2, O] fp16
  out        [S, O]  fp32   partial output

Attention works in transposed-score space: sT[k, q] = KT_slice.T @ QT (two
heads packed into PE row-groups 0-63 / 64-127), one exp on ACT covers both
heads, then y'T = V'.T @ PT where V' carries a ones column so row 64 of y'T
accumulates the softmax denominator (scores are O(1), so max-subtraction is
unnecessary). The denominator row is broadcast across partitions on the
otherwise-idle GPSIMD engine, reciprocated on DVE, and multiplied into fp16
yT tiles used as lhsT of the output projection.

Schedule: the ACT engine's exp stream (1057ns per k-tile) is slower than the
PE's per-k-tile work in the attention phase, while the projection phase is
the reverse (PE busy, ACT idle). So scores+exp for the first TWO blocks are
emitted greedily inside the projection phase (as each K chunk lands), giving
ACT a 30us head start; during block bi's y-loop the scores for block bi+2
stream (2-block exp cushion). x DMAs ride two rings (xq on SP, xkv on
GPSIMD) split into half-chunks so the first projection matmul starts ~2us
after kernel start, with dummy matmuls keeping the PE p-state ramp warm
through the initial DMA window. Normalization + output-projection work is
spread through later kt iterations in sub-microsecond units so the PE fills
the slack the ACT-gated attention phase would otherwise leave.
"""

import numpy as np

import concourse.bass as bass
import concourse.mybir as mybir
import concourse.tile as tile
from concourse import bacc
from concourse.bass_utils import run_bass_kernel_spmd

F32 = mybir.dt.float32
F32R = mybir.dt.float32r
F16 = mybir.dt.float16
AF = mybir.ActivationFunctionType

B, S, F, H, DH, O = 2, 2048, 1024, 16, 64, 1024
NCORES = 8
HPC = 4  # heads per core
CH = 512  # q-chunk width
P = 128


def build_program(s=S, f=F, o=O, hpc=HPC):
    npair = hpc // 2
    nch = s // CH  # q chunks
    nkt = s // P  # k tiles
    nf = f // P  # contraction tiles for projections
    nfh = nf // 2  # f-tiles per half-chunk DMA
    hd = hpc * DH  # stacked head dims per core (256)

    nc = bacc.Bacc("TRN2", target_bir_lowering=False, debug=False, num_devices=NCORES)

    xqT = nc.dram_tensor("xqT", [P, nch, nf, CH], F16, kind="ExternalInput")
    xkvT = nc.dram_tensor("xkvT", [P, nch, nf, CH], F16, kind="ExternalInput")
    wq = nc.dram_tensor("wq", [P, nf, hd], F16, kind="ExternalInput")
    wk = nc.dram_tensor("wk", [P, nf, hd], F16, kind="ExternalInput")
    wv = nc.dram_tensor("wv", [P, nf, hd], F16, kind="ExternalInput")
    wo = nc.dram_tensor("wo", [P, hd // P, o], F16, kind="ExternalInput")
    out = nc.dram_tensor("out", [s, o], F32, kind="ExternalOutput")

    xqT_t = xqT.ap()  # [128, nch, nf, CH]
    xkvT_t = xkvT.ap()

    with tile.TileContext(nc) as tc:
        with (
            tc.tile_pool(name="weights", bufs=1) as wpool,
            tc.tile_pool(name="xin", bufs=2) as xpool,
            tc.tile_pool(name="qkv", bufs=1) as qkvpool,
            tc.tile_pool(name="pt", bufs=1) as ptpool,
            tc.tile_pool(name="norm", bufs=4) as npool,
            tc.tile_pool(name="outsb", bufs=2) as opool,
        ):
            # ---- weights + constants -------------------------------------
            # weight DMAs ride the ACT HWDGE ring so they don't head-of-line
            # block the x streams; packed layout = one 4KB run per partition
            wq_sb = wpool.tile([P, nf, hd], F16, tag="wq")
            wk_sb = wpool.tile([P, nf, hd], F16, tag="wk")
            wv_sb = wpool.tile([P, nf, hd], F16, tag="wv")
            wo_sb = wpool.tile([P, hd // P, o], F16, tag="wo")
            nc.scalar.dma_start(wq_sb[:], wq.ap())
            nc.scalar.dma_start(wk_sb[:], wk.ap())
            # wv/wo DMAs are deferred into the chunk loop (wv needed ~20us
            # in, wo ~85us in) so chunk 0's x stream gets full HBM bandwidth
            # memset can't write fp16/fp32r; memset fp32 scratch, cast-copy
            ones_f32 = wpool.tile([P, 4 * P], F32, tag="ones_f32")
            nc.vector.memset(ones_f32[:], 1.0)
            ones_sb = wpool.tile([1, 4 * P], F16, tag="ones")
            nc.vector.tensor_copy(ones_sb[:], ones_f32[0:1, :])

            # ---- storage -------------------------------------------------
            QT = [
                [qkvpool.tile([P, CH], F16, tag=f"QT{p_}_{c}", name=f"QT{p_}_{c}") for c in range(nch)]
                for p_ in range(npair)
            ]
            KT = [
                [qkvpool.tile([P, CH], F16, tag=f"KT{p_}_{c}", name=f"KT{p_}_{c}") for c in range(nch)]
                for p_ in range(npair)
            ]
            # V': per k-tile [128, hpc, DH+1]; last column is ones
            V = [qkvpool.tile([P, hpc, DH + 1], F16, tag=f"V{kt}", name=f"V{kt}") for kt in range(nkt)]
            YT = [
                [qkvpool.tile([P, CH], F16, tag=f"YT{p_}_{c}", name=f"YT{p_}_{c}") for c in range(nch)]
                for p_ in range(npair)
            ]
            for kt in range(nkt):
                nc.vector.tensor_copy(V[kt][:, :, DH], ones_f32[:, 0:hpc])

            # ps_s (scores PSUM) lives for the whole kernel: 4 banks.
            # Projection-phase pools add 4 more (within the 8-bank budget);
            # after they close, the psY/ps_o pools take the 4.
            with tc.tile_pool(name="ps_att", bufs=2, space="PSUM") as ps_att:
                blocks = [(c, p_) for c in range(nch) for p_ in range(npair)]

                def emit_scores(p_, c, kt):
                    ps_s = ps_att.tile([P, 2 * CH], F32, tag="ps_s", name="ps_s")
                    nc.tensor.matmul(
                        ps_s[:, 0:CH],
                        KT[p_][kt // 4][0:DH, (kt % 4) * P : (kt % 4 + 1) * P],
                        QT[p_][c][0:DH, :],
                        tile_position=(0, 0),
                    )
                    nc.tensor.matmul(
                        ps_s[:, CH : 2 * CH],
                        KT[p_][kt // 4][DH : 2 * DH, (kt % 4) * P : (kt % 4 + 1) * P],
                        QT[p_][c][DH : 2 * DH, :],
                        tile_position=(DH, 0),
                    )
                    return ps_s

                # saved exp(scores) tiles; (block parity, kt) keys the SBUF
                # slot. kt<8 gets THREE parities so block 2's first-half
                # scores can also run under the projections (extra ACT work
                # pulled out of the ACT-bound attention phase) and the Y-loop
                # keeps a deeper emission horizon; kt>=8 keeps two parities
                # to bound SBUF.
                PT = {}

                def pt_par(bi, kt):
                    return bi % 3 if kt < 8 else bi % 2

                def emit_score_exp(bi, kt):
                    c, p_ = blocks[bi]
                    ps_s = emit_scores(p_, c, kt)
                    par = pt_par(bi, kt)
                    pt = ptpool.tile(
                        [P, 2 * CH], F16, tag=f"pt{par}_{kt}", name=f"pt{par}_{kt}"
                    )
                    nc.scalar.activation(pt[:], ps_s[:], AF.Exp)
                    PT[(bi, kt)] = pt

                # ---- projections (+ early scores/exp interleaved) ---------
                # the PE is in-order, so ACT-gated score pairs must be SPREAD
                # between projection matmuls at the ACT consumption rate
                # (~1.06us/pair) or they head-of-line block the projections.
                # score_q holds enabled (block, kt) pairs; pe_tick() pops one
                # pair per ~750ns of other PE work (pair itself adds ~320ns).
                score_q = []
                queued = set()
                credit = [0.0]

                def enqueue_scores(bi, kt):
                    score_q.append((bi, kt))
                    queued.add((bi, kt))

                def pop_scores():
                    # always emit the most urgently-consumed pair first
                    # (lowest block, then k-tile): a deep-horizon pair must
                    # never delay a near-horizon one on the in-order ACT
                    i = min(range(len(score_q)), key=lambda ix: score_q[ix])
                    return score_q.pop(i)

                def pe_tick(ns):
                    credit[0] += ns
                    if not score_q:
                        credit[0] = min(credit[0], 750.0)
                        return
                    if credit[0] >= 750.0:
                        credit[0] -= 750.0
                        emit_score_exp(*pop_scores())

                with (
                    tc.tile_pool(name="ps_projqk", bufs=1, space="PSUM") as ps_projqk,
                    tc.tile_pool(name="ps_projv", bufs=2, space="PSUM") as ps_projv,
                ):
                    # PE warm-up: dummy matmuls with no DMA dependency keep
                    # the PE busy through the first x DMA window so the real
                    # projection matmuls start at 2.4GHz instead of 1.2GHz
                    for wu in range(28):
                        ps_wu = ps_projv.tile([P, P], F32, tag="psV", name="ps_wu")
                        nc.tensor.matmul(ps_wu[:], ones_sb[0:1, 0:P], ones_sb[0:1, 0:P])
                    nq = 4  # x DMA split per chunk per stream
                    nfq = nf // nq
                    xkv_store = {}
                    for c in range(nch):
                        # quarter-chunk DMAs on the SP ring: fine granularity
                        # so the first projection matmul starts ~1us after
                        # the first quarter lands
                        # DMA issue order matches pass order: chunk 0 runs
                        # Q first (needs xq), later chunks run K first
                        xq_h = []
                        xkv_h = []

                        def dma_xq():
                            for hf in range(nq):
                                t_ = xpool.tile([P, nfq, CH], F16, tag=f"xq{hf}", name=f"xq{hf}")
                                nc.sync.dma_start(
                                    t_[:], xqT_t[:, c, hf * nfq : (hf + 1) * nfq]
                                )
                                xq_h.append(t_)

                        def dma_xkv():
                            for hf in range(nq):
                                t_ = xpool.tile([P, nfq, CH], F16, tag=f"xkv{hf}", name=f"xkv{hf}")
                                nc.sync.dma_start(
                                    t_[:], xkvT_t[:, c, hf * nfq : (hf + 1) * nfq]
                                )
                                xkv_h.append(t_)

                        dma_xq()
                        dma_xkv()
                        # deferred weight DMAs: wv before chunk 0's V pass,
                        # wo well ahead of the first output projection
                        if c == 0:
                            nc.scalar.dma_start(wv_sb[:], wv.ap())
                        elif c == 1:
                            nc.scalar.dma_start(wo_sb[:], wo.ap())
                        def q_pass():
                            psQ = [ps_projqk.tile([P, CH], F32, tag=f"psQK{m}", name="psQ") for m in range(npair)]
                            for ft in range(nf):
                                for m in range(npair):
                                    nc.tensor.matmul(
                                        psQ[m][:],
                                        wq_sb[:, ft, m * P : (m + 1) * P],
                                        xq_h[ft // nfq][:, ft % nfq],
                                        start=(ft == 0),
                                        stop=(ft == nf - 1),
                                    )
                                    pe_tick(216)
                            for m in range(npair):
                                nc.vector.tensor_copy(QT[m][c][:], psQ[m][:])

                        def k_pass():
                            psK = [ps_projqk.tile([P, CH], F32, tag=f"psQK{m}", name="psK") for m in range(npair)]
                            for ft in range(nf):
                                for m in range(npair):
                                    nc.tensor.matmul(
                                        psK[m][:],
                                        wk_sb[:, ft, m * P : (m + 1) * P],
                                        xkv_h[ft // nfq][:, ft % nfq],
                                        start=(ft == 0),
                                        stop=(ft == nf - 1),
                                    )
                                    pe_tick(216)
                            for m in range(npair):
                                nc.vector.tensor_copy(KT[m][c][:], psK[m][:])

                        def enable_chunk_scores():
                            # blocks 0/1 for this chunk's k-tiles, plus
                            # block 2's first-half scores one chunk behind
                            # (its Q needs chunk 1); pe_tick interleaves
                            # them at the ACT rate
                            for kt in range(4 * c, 4 * c + 4):
                                enqueue_scores(0, kt)
                                enqueue_scores(1, kt)
                            if c >= 1:
                                for kt in range(4 * (c - 1), min(4 * c, 8)):
                                    enqueue_scores(2, kt)

                        xkv_store[c] = xkv_h
                        q_pass()
                        k_pass()
                        enable_chunk_scores()

                        def v_pass(cv):
                            # V pass (xkv chunk tile as lhsT); one PSUM
                            # accumulation group per bank, so st is outer
                            xk = xkv_store[cv]
                            for st in range(4):
                                psV = ps_projv.tile([P, CH], F32, tag="psV", name="psV")
                                for ft in range(nf):
                                    nc.tensor.matmul(
                                        psV[:, 0:hd],
                                        xk[ft // nfq][:, ft % nfq, st * P : (st + 1) * P],
                                        wv_sb[:, ft, :],
                                        start=(ft == 0),
                                        stop=(ft == nf - 1),
                                    )
                                    pe_tick(108)
                                kt = cv * 4 + st
                                nc.vector.tensor_copy(
                                    V[kt][:, :, 0:DH],
                                    psV[:, 0:hd].rearrange("p (h d) -> p h d", h=hpc),
                                )

                        # chunk 0's V pass is deferred past chunk 1's K pass
                        # and score enablement: V isn't needed until the
                        # attention phase, and sliding it out of the critical
                        # window closes the ~4us ACT dry-out at the chunk 0->1
                        # boundary (V tiles still land long before their y's)
                        if c == 1:
                            v_pass(0)
                            v_pass(1)
                        elif c >= 2:
                            v_pass(c)

                # deferred work queue: sub-microsecond PE units injected into
                # later kt iterations so the PE fills the slack the ACT-gated
                # attention phase leaves
                pending = []

                def queue_normalize(p_, c, psY, last=False):
                    def emit(h01, psY=psY):
                        # copy the 1-row denominator to SBUF, reciprocate it
                        # there (the approx-fast bit tricks need SBUF fp32,
                        # not raw PSUM), THEN broadcast the inverse on the
                        # idle GPSIMD engine: no PE matmul, no ps_s PSUM-slot
                        # churn, 3x less DVE time than broadcast-then-recip
                        den_r = npool.tile([1, CH], F32, tag="den", name="den_r")
                        nc.vector.tensor_copy(den_r[:], psY[h01][DH : DH + 1, :])
                        inv_r = npool.tile([1, CH], F32, tag="invr", name="inv_r")
                        nc.vector.reciprocal_approx_fast(out=inv_r[:], in_=den_r[:])
                        inv_sb = npool.tile([DH, CH], F32, tag="inv", name="inv_sb")
                        nc.gpsimd.partition_broadcast(inv_sb[:], inv_r[:])
                        nc.vector.tensor_tensor(
                            YT[p_][c][h01 * DH : (h01 + 1) * DH, :],
                            psY[h01][0:DH, :],
                            inv_sb[:],
                            mybir.AluOpType.mult,
                        )

                    # emit immediately: normalize is DVE/GPSIMD-only work (no
                    # PE time), and early emission frees the psY banks sooner
                    # for the next block's accumulation
                    emit(0)
                    emit(1)

                def queue_outproj(c):
                    # single-matmul granularity: each pending unit costs the
                    # PE at most 216ns, under the ~280ns/kt slack the
                    # ACT-paced attention loop leaves, so pops never slip the
                    # exp cadence (copies ride DVE, DMAs ride SP: PE-free)
                    for st in range(4):
                        qt = c * 4 + st
                        carrier = {}

                        def emit_mm(j, m, st=st, c=c, carrier=carrier):
                            if j == 0 and m == 0:
                                carrier["out_sb"] = opool.tile([P, o], F32, tag="out_sb", name="out_sb")
                            if m == 0:
                                carrier["ps_o"] = ps_opool.tile([P, CH], F32, tag="ps_o", name="ps_o")
                            nc.tensor.matmul(
                                carrier["ps_o"][:],
                                YT[m][c][:, st * P : (st + 1) * P],
                                wo_sb[:, m, j * CH : (j + 1) * CH],
                                start=(m == 0),
                                stop=(m == hd // P - 1),
                            )

                        def emit_copy(j, carrier=carrier):
                            nc.vector.tensor_copy(
                                carrier["out_sb"][:, j * CH : (j + 1) * CH],
                                carrier["ps_o"][:],
                            )

                        def emit_dma(qt=qt, carrier=carrier):
                            nc.sync.dma_start(
                                out.ap()[qt * P : (qt + 1) * P, :], carrier["out_sb"][:]
                            )

                        for j in range(2):
                            for m in range(hd // P):
                                pending.append(lambda j=j, m=m, f=emit_mm: f(j, m))
                            pending.append(lambda j=j, f=emit_copy: f(j))
                        pending.append(emit_dma)

                # block-level pipeline: during block bi's y-phase, block
                # bi+2's scores/exp stream on ACT (blocks 0/1 ran under the
                # projections), so y-matmuls run with a 2-block exp cushion
                with (
                    tc.tile_pool(name="ps_y0", bufs=2, space="PSUM") as ps_y0pool,
                    tc.tile_pool(name="ps_y1", bufs=1, space="PSUM") as ps_y1pool,
                    tc.tile_pool(name="ps_o", bufs=1, space="PSUM") as ps_opool,
                ):
                    for bi, (c, p_) in enumerate(blocks):
                        hA, hB = 2 * p_, 2 * p_ + 1
                        psY0 = ps_y0pool.tile([DH + 1, CH], F32, tag="psY0", name="psY0")
                        if bi == len(blocks) - 1:
                            # the last block's head-B accumulator borrows the
                            # (now drained) outproj bank: no wait on
                            # normalize(bi-1, h1) reading the single psY1
                            psY1f = ps_opool.tile([P, CH], F32, tag="ps_o", name="psY1f")
                            psY = [psY0, psY1f[0 : DH + 1]]
                        else:
                            psY = [
                                psY0,
                                ps_y1pool.tile([DH + 1, CH], F32, tag="psY1", name="psY1"),
                            ]
                        for kt2 in range(0, nkt, 2):
                            # y for two k-tiles first (frees both pt slots),
                            # then both score pairs back-to-back: the second
                            # pair skips the PE tile-config switch penalty
                            for kt in (kt2, kt2 + 1):
                                pt = PT.pop((bi, kt))
                                nc.tensor.matmul(
                                    psY[0][:],
                                    V[kt][:, hA, :],
                                    pt[:, 0:CH],
                                    start=(kt == 0),
                                    stop=(kt == nkt - 1),
                                )
                                nc.tensor.matmul(
                                    psY[1][:],
                                    V[kt][:, hB, :],
                                    pt[:, CH : 2 * CH],
                                    start=(kt == 0),
                                    stop=(kt == nkt - 1),
                                )
                            for kt in (kt2, kt2 + 1):
                                if bi + 2 < len(blocks) and (bi + 2, kt) not in queued:
                                    enqueue_scores(bi + 2, kt)
                                if score_q:
                                    emit_score_exp(*pop_scores())
                            npop = 4 if (bi >= len(blocks) - 2 or len(pending) > 16) else 2
                            for _ in range(npop):
                                if pending:
                                    pending.pop(0)()
                        queue_normalize(p_, c, psY, last=(bi == len(blocks) - 1))
                        if p_ == npair - 1 and c < nch - 1:
                            queue_outproj(c)
                    while pending:
                        pending.pop(0)()

                # last chunk's output projection runs after the psY pools
                # close, with a 4-deep PSUM rotation and copies alternating
                # between DVE and the now-idle ACT engine, so the drain isn't
                # serialized on a single ps_o bank
                with tc.tile_pool(name="ps_od", bufs=4, space="PSUM") as ps_odpool:
                    c = nch - 1
                    for st in range(4):
                        out_sb = opool.tile([P, o], F32, tag="out_sb", name="out_sb")
                        for j in range(2):
                            ps_o = ps_odpool.tile([P, CH], F32, tag="ps_od", name="ps_od")
                            for m in range(hd // P):
                                nc.tensor.matmul(
                                    ps_o[:],
                                    YT[m][c][:, st * P : (st + 1) * P],
                                    wo_sb[:, m, j * CH : (j + 1) * CH],
                                    start=(m == 0),
                                    stop=(m == hd // P - 1),
                                )
                            if (2 * st + j) % 2 == 0:
                                nc.vector.tensor_copy(
                                    out_sb[:, j * CH : (j + 1) * CH], ps_o[:]
                                )
                            else:
                                nc.scalar.copy(
                                    out_sb[:, j * CH : (j + 1) * CH], ps_o[:]
                                )
                        qt = c * 4 + st
                        nc.sync.dma_start(
                            out.ap()[qt * P : (qt + 1) * P, :], out_sb[:]
                        )

    nc.compile()
    return nc


def _pack_x(xT):
    """[F, S] -> [128, nch, nf, CH] with [p, c, t, n] = xT[t*128+p, c*CH+n],
    so each (partition, chunk) is one contiguous multi-KB DMA run."""
    nf, nch = F // P, S // CH
    return np.ascontiguousarray(
        xT.reshape(nf, P, nch, CH).transpose(1, 2, 0, 3)
    ).astype(np.float16)


def _pack_w(w2d):
    """[F, hd] -> [128, nf, hd] with [p, t, :] = w2d[t*128+p, :]."""
    nf = F // P
    hd = w2d.shape[1]
    return np.ascontiguousarray(w2d.reshape(nf, P, hd).transpose(1, 0, 2)).astype(
        np.float16
    )


def make_in_maps(inputs_q, inputs_kv, wq, wk, wv, wo):
    """Shard full inputs into 8 per-core input dicts (host-side)."""
    in_maps = []
    scale = 1.0 / np.sqrt(DH)
    hd = HPC * DH
    for core in range(NCORES):
        b = core // (NCORES // B)
        hg = core % (NCORES // B)
        hs = slice(hg * HPC, (hg + 1) * HPC)
        in_maps.append(
            {
                "xqT": _pack_x(inputs_q[b].T),
                "xkvT": _pack_x(inputs_kv[b].T),
                "wq": _pack_w((wq[:, hs, :] * scale).reshape(F, hd)),
                "wk": _pack_w(wk[:, hs, :].reshape(F, hd)),
                "wv": _pack_w(wv[:, hs, :].reshape(F, hd)),
                "wo": np.ascontiguousarray(
                    wo[hs].reshape(hd, O).reshape(hd // P, P, O).transpose(1, 0, 2)
                ).astype(np.float16),
            }
        )
    return in_maps


_CACHE = {}


def _get_program():
    if "nc" not in _CACHE:
        _CACHE["nc"] = build_program()
    return _CACHE["nc"]


def run_sharded(inputs_q, inputs_kv, wq, wk, wv, wo, bo, **spmd_kwargs):
    """Build in_maps, run on 8 cores, reduce partials. Returns (out, results)."""
    nc = _get_program()
    in_maps = make_in_maps(inputs_q, inputs_kv, wq, wk, wv, wo)
    res = run_bass_kernel_spmd(nc, in_maps, core_ids=list(range(NCORES)), **spmd_kwargs)
    gpb = NCORES // B  # head-group cores per batch element
    out = np.zeros((B, S, O), dtype=np.float32)
    for core in range(NCORES):
        out[core // gpb] += res.results[core]["out"]
    out += np.asarray(bo, dtype=np.float32)
    return out, res


def kernel(inputs_q, inputs_kv, wq, wk, wv, wo, bo):
    out, _ = run_sharded(
        np.asarray(inputs_q),
        np.asarray(inputs_kv),
        np.asarray(wq),
        np.asarray(wk),
        np.asarray(wv),
        np.asarray(wo),
        np.asarray(bo),
    )
    return out


# revision 51
# speedup vs baseline: 1.0104x; 1.0104x over previous
"""Multi-head dot-product attention (B=2, S=2048, F=1024, H=16, DH=64, O=1024)
as a Bass/Tile kernel on 8 Trainium2 NeuronCores.

Sharding: data-parallel over B (2) x tensor-parallel over H (4 groups of 4
heads) = 8 cores. Each core computes q/k/v projections for its 4 heads,
softmax attention, and a partial output projection; the host sums the 4
partial outputs per batch element and adds the bias.

Device layouts (per core, host-pre-packed so every DMA reads multi-KB
contiguous runs per partition instead of 1KB strided rows):
  xqT, xkvT  [128, nch, nf, CH] fp16  = xT[t*128+p, c*CH+n] at [p,c,t,n]
  wq, wk, wv [128, nf, 4*DH] fp16     (wq pre-scaled by 1/sqrt(DH))
  wo         [128, 2, O] fp16
  out        [S, O]  fp32   partial output

Attention works in transposed-score space: sT[k, q] = KT_slice.T @ QT (two
heads packed into PE row-groups 0-63 / 64-127), one exp on ACT covers both
heads, then y'T = V'.T @ PT where V' carries a ones column so row 64 of y'T
accumulates the softmax denominator (scores are O(1), so max-subtraction is
unnecessary). The denominator row is broadcast across partitions on the
otherwise-idle GPSIMD engine, reciprocated on DVE, and multiplied into fp16
yT tiles used as lhsT of the output projection.

Schedule: the ACT engine's exp stream (1057ns per k-tile) is slower than the
PE's per-k-tile work in the attention phase, while the projection phase is
the reverse (PE busy, ACT idle). So scores+exp for the first TWO blocks are
emitted greedily inside the projection phase (as each K chunk lands), giving
ACT a 30us head start; during block bi's y-loop the scores for block bi+2
stream (2-block exp cushion). x DMAs ride two rings (xq on SP, xkv on
GPSIMD) split into half-chunks so the first projection matmul starts ~2us
after kernel start, with dummy matmuls keeping the PE p-state ramp warm
through the initial DMA window. Normalization + output-projection work is
spread through later kt iterations in sub-microsecond units so the PE fills
the slack the ACT-gated attention phase would otherwise leave.
"""

import numpy as np

import concourse.bass as bass
import concourse.mybir as mybir
import concourse.tile as tile
from concourse import bacc
from concourse.bass_utils import run_bass_kernel_spmd

F32 = mybir.dt.float32
F32R = mybir.dt.float32r
F16 = mybir.dt.float16
AF = mybir.ActivationFunctionType

B, S, F, H, DH, O = 2, 2048, 1024, 16, 64, 1024
NCORES = 8
HPC = 4  # heads per core
CH = 512  # q-chunk width
P = 128


def build_program(s=S, f=F, o=O, hpc=HPC):
    npair = hpc // 2
    nch = s // CH  # q chunks
    nkt = s // P  # k tiles
    nf = f // P  # contraction tiles for projections
    nfh = nf // 2  # f-tiles per half-chunk DMA
    hd = hpc * DH  # stacked head dims per core (256)

    nc = bacc.Bacc("TRN2", target_bir_lowering=False, debug=False, num_devices=NCORES)

    xqT = nc.dram_tensor("xqT", [P, nch, nf, CH], F16, kind="ExternalInput")
    xkvT = nc.dram_tensor("xkvT", [P, nch, nf, CH], F16, kind="ExternalInput")
    wq = nc.dram_tensor("wq", [P, nf, hd], F16, kind="ExternalInput")
    wk = nc.dram_tensor("wk", [P, nf, hd], F16, kind="ExternalInput")
    wv = nc.dram_tensor("wv", [P, nf, hd], F16, kind="ExternalInput")
    wo = nc.dram_tensor("wo", [P, hd // P, o], F16, kind="ExternalInput")
    out = nc.dram_tensor("out", [s, o], F32, kind="ExternalOutput")

    xqT_t = xqT.ap()  # [128, nch, nf, CH]
    xkvT_t = xkvT.ap()

    with tile.TileContext(nc) as tc:
        with (
            tc.tile_pool(name="weights", bufs=1) as wpool,
            tc.tile_pool(name="xin", bufs=2) as xpool,
            tc.tile_pool(name="qkv", bufs=1) as qkvpool,
            tc.tile_pool(name="pt", bufs=1) as ptpool,
            tc.tile_pool(name="norm", bufs=4) as npool,
            tc.tile_pool(name="outsb", bufs=2) as opool,
        ):
            # ---- weights + constants -------------------------------------
            # weight DMAs ride the ACT HWDGE ring so they don't head-of-line
            # block the x streams; packed layout = one 4KB run per partition
            wq_sb = wpool.tile([P, nf, hd], F16, tag="wq")
            wk_sb = wpool.tile([P, nf, hd], F16, tag="wk")
            wv_sb = wpool.tile([P, nf, hd], F16, tag="wv")
            wo_sb = wpool.tile([P, hd // P, o], F16, tag="wo")
            nc.scalar.dma_start(wq_sb[:], wq.ap())
            nc.scalar.dma_start(wk_sb[:], wk.ap())
            # wv/wo DMAs are deferred into the chunk loop (wv needed ~20us
            # in, wo ~85us in) so chunk 0's x stream gets full HBM bandwidth
            # memset can't write fp16/fp32r; memset fp32 scratch, cast-copy
            ones_f32 = wpool.tile([P, 4 * P], F32, tag="ones_f32")
            nc.vector.memset(ones_f32[:], 1.0)
            ones_sb = wpool.tile([1, 4 * P], F16, tag="ones")
            nc.vector.tensor_copy(ones_sb[:], ones_f32[0:1, :])

            # ---- storage -------------------------------------------------
            QT = [
                [qkvpool.tile([P, CH], F16, tag=f"QT{p_}_{c}", name=f"QT{p_}_{c}") for c in range(nch)]
                for p_ in range(npair)
            ]
            KT = [
                [qkvpool.tile([P, CH], F16, tag=f"KT{p_}_{c}", name=f"KT{p_}_{c}") for c in range(nch)]
                for p_ in range(npair)
            ]
            # V': per k-tile [128, hpc, DH+1]; last column is ones
            V = [qkvpool.tile([P, hpc, DH + 1], F16, tag=f"V{kt}", name=f"V{kt}") for kt in range(nkt)]
            YT = [
                [qkvpool.tile([P, CH], F16, tag=f"YT{p_}_{c}", name=f"YT{p_}_{c}") for c in range(nch)]
                for p_ in range(npair)
            ]
            for kt in range(nkt):
                nc.vector.tensor_copy(V[kt][:, :, DH], ones_f32[:, 0:hpc])

            # ps_s (scores PSUM) lives for the whole kernel: 4 banks.
            # Projection-phase pools add 4 more (within the 8-bank budget);
            # after they close, the psY/ps_o pools take the 4.
            with tc.tile_pool(name="ps_att", bufs=2, space="PSUM") as ps_att:
                blocks = [(c, p_) for c in range(nch) for p_ in range(npair)]

                def emit_scores(p_, c, kt):
                    ps_s = ps_att.tile([P, 2 * CH], F32, tag="ps_s", name="ps_s")
                    nc.tensor.matmul(
                        ps_s[:, 0:CH],
                        KT[p_][kt // 4][0:DH, (kt % 4) * P : (kt % 4 + 1) * P],
                        QT[p_][c][0:DH, :],
                        tile_position=(0, 0),
                    )
                    nc.tensor.matmul(
                        ps_s[:, CH : 2 * CH],
                        KT[p_][kt // 4][DH : 2 * DH, (kt % 4) * P : (kt % 4 + 1) * P],
                        QT[p_][c][DH : 2 * DH, :],
                        tile_position=(DH, 0),
                    )
                    return ps_s

                # saved exp(scores) tiles; (block parity, kt) keys the SBUF
                # slot. kt<8 gets THREE parities so block 2's first-half
                # scores can also run under the projections (extra ACT work
                # pulled out of the ACT-bound attention phase) and the Y-loop
                # keeps a deeper emission horizon; kt>=8 keeps two parities
                # to bound SBUF.
                PT = {}

                def pt_par(bi, kt):
                    return bi % 3 if kt < 8 else bi % 2

                def emit_score_exp(bi, kt):
                    c, p_ = blocks[bi]
                    ps_s = emit_scores(p_, c, kt)
                    par = pt_par(bi, kt)
                    pt = ptpool.tile(
                        [P, 2 * CH], F16, tag=f"pt{par}_{kt}", name=f"pt{par}_{kt}"
                    )
                    nc.scalar.activation(pt[:], ps_s[:], AF.Exp)
                    PT[(bi, kt)] = pt

                # ---- projections (+ early scores/exp interleaved) ---------
                # the PE is in-order, so ACT-gated score pairs must be SPREAD
                # between projection matmuls at the ACT consumption rate
                # (~1.06us/pair) or they head-of-line block the projections.
                # score_q holds enabled (block, kt) pairs; pe_tick() pops one
                # pair per ~750ns of other PE work (pair itself adds ~320ns).
                score_q = []
                queued = set()
                credit = [0.0]

                def enqueue_scores(bi, kt):
                    score_q.append((bi, kt))
                    queued.add((bi, kt))

                def pop_scores():
                    # always emit the most urgently-consumed pair first
                    # (lowest block, then k-tile): a deep-horizon pair must
                    # never delay a near-horizon one on the in-order ACT
                    i = min(range(len(score_q)), key=lambda ix: score_q[ix])
                    return score_q.pop(i)

                def pe_tick(ns):
                    credit[0] += ns
                    if not score_q:
                        credit[0] = min(credit[0], 750.0)
                        return
                    if credit[0] >= 750.0:
                        credit[0] -= 750.0
                        emit_score_exp(*pop_scores())

                with (
                    tc.tile_pool(name="ps_projqk", bufs=1, space="PSUM") as ps_projqk,
                    tc.tile_pool(name="ps_projv", bufs=2, space="PSUM") as ps_projv,
                ):
                    # PE warm-up: dummy matmuls with no DMA dependency keep
                    # the PE busy through the first x DMA window so the real
                    # projection matmuls start at 2.4GHz instead of 1.2GHz
                    for wu in range(28):
                        ps_wu = ps_projv.tile([P, P], F32, tag="psV", name="ps_wu")
                        nc.tensor.matmul(ps_wu[:], ones_sb[0:1, 0:P], ones_sb[0:1, 0:P])
                    nq = 4  # x DMA split per chunk per stream
                    nfq = nf // nq
                    for c in range(nch):
                        # quarter-chunk DMAs on the SP ring: fine granularity
                        # so the first projection matmul starts ~1us after
                        # the first quarter lands
                        # DMA issue order matches pass order: chunk 0 runs
                        # Q first (needs xq), later chunks run K first
                        xq_h = []
                        xkv_h = []

                        def dma_xq():
                            for hf in range(nq):
                                t_ = xpool.tile([P, nfq, CH], F16, tag=f"xq{hf}", name=f"xq{hf}")
                                nc.sync.dma_start(
                                    t_[:], xqT_t[:, c, hf * nfq : (hf + 1) * nfq]
                                )
                                xq_h.append(t_)

                        def dma_xkv():
                            for hf in range(nq):
                                t_ = xpool.tile([P, nfq, CH], F16, tag=f"xkv{hf}", name=f"xkv{hf}")
                                nc.sync.dma_start(
                                    t_[:], xkvT_t[:, c, hf * nfq : (hf + 1) * nfq]
                                )
                                xkv_h.append(t_)

                        dma_xq()
                        dma_xkv()
                        # deferred weight DMAs: wv before chunk 0's V pass,
                        # wo well ahead of the first output projection
                        if c == 0:
                            nc.scalar.dma_start(wv_sb[:], wv.ap())
                        elif c == 1:
                            nc.scalar.dma_start(wo_sb[:], wo.ap())
                        def q_pass():
                            psQ = [ps_projqk.tile([P, CH], F32, tag=f"psQK{m}", name="psQ") for m in range(npair)]
                            for ft in range(nf):
                                for m in range(npair):
                                    nc.tensor.matmul(
                                        psQ[m][:],
                                        wq_sb[:, ft, m * P : (m + 1) * P],
                                        xq_h[ft // nfq][:, ft % nfq],
                                        start=(ft == 0),
                                        stop=(ft == nf - 1),
                                    )
                                    pe_tick(216)
                            for m in range(npair):
                                nc.vector.tensor_copy(QT[m][c][:], psQ[m][:])

                        def k_pass():
                            psK = [ps_projqk.tile([P, CH], F32, tag=f"psQK{m}", name="psK") for m in range(npair)]
                            for ft in range(nf):
                                for m in range(npair):
                                    nc.tensor.matmul(
                                        psK[m][:],
                                        wk_sb[:, ft, m * P : (m + 1) * P],
                                        xkv_h[ft // nfq][:, ft % nfq],
                                        start=(ft == 0),
                                        stop=(ft == nf - 1),
                                    )
                                    pe_tick(216)
                            for m in range(npair):
                                nc.vector.tensor_copy(KT[m][c][:], psK[m][:])

                        def enable_chunk_scores():
                            # blocks 0/1 for this chunk's k-tiles, plus
                            # block 2's first-half scores one chunk behind
                            # (its Q needs chunk 1); pe_tick interleaves
                            # them at the ACT rate
                            for kt in range(4 * c, 4 * c + 4):
                                enqueue_scores(0, kt)
                                enqueue_scores(1, kt)
                            if c >= 1:
                                for kt in range(4 * (c - 1), min(4 * c, 8)):
                                    enqueue_scores(2, kt)

                        q_pass()
                        k_pass()
                        enable_chunk_scores()
                        # V pass (xkv chunk tile as lhsT); one PSUM
                        # accumulation group per bank, so st is outer
                        for st in range(4):
                            psV = ps_projv.tile([P, CH], F32, tag="psV", name="psV")
                            for ft in range(nf):
                                nc.tensor.matmul(
                                    psV[:, 0:hd],
                                    xkv_h[ft // nfq][:, ft % nfq, st * P : (st + 1) * P],
                                    wv_sb[:, ft, :],
                                    start=(ft == 0),
                                    stop=(ft == nf - 1),
                                )
                                pe_tick(108)
                            kt = c * 4 + st
                            nc.vector.tensor_copy(
                                V[kt][:, :, 0:DH],
                                psV[:, 0:hd].rearrange("p (h d) -> p h d", h=hpc),
                            )

                # deferred work queue: sub-microsecond PE units injected into
                # later kt iterations so the PE fills the slack the ACT-gated
                # attention phase leaves
                pending = []

                def queue_normalize(p_, c, psY, last=False):
                    def emit(h01, psY=psY):
                        # copy the 1-row denominator to SBUF, reciprocate it
                        # there (the approx-fast bit tricks need SBUF fp32,
                        # not raw PSUM), THEN broadcast the inverse on the
                        # idle GPSIMD engine: no PE matmul, no ps_s PSUM-slot
                        # churn, 3x less DVE time than broadcast-then-recip
                        den_r = npool.tile([1, CH], F32, tag="den", name="den_r")
                        nc.vector.tensor_copy(den_r[:], psY[h01][DH : DH + 1, :])
                        inv_r = npool.tile([1, CH], F32, tag="invr", name="inv_r")
                        nc.vector.reciprocal_approx_fast(out=inv_r[:], in_=den_r[:])
                        inv_sb = npool.tile([DH, CH], F32, tag="inv", name="inv_sb")
                        nc.gpsimd.partition_broadcast(inv_sb[:], inv_r[:])
                        nc.vector.tensor_tensor(
                            YT[p_][c][h01 * DH : (h01 + 1) * DH, :],
                            psY[h01][0:DH, :],
                            inv_sb[:],
                            mybir.AluOpType.mult,
                        )

                    # emit immediately: normalize is DVE/GPSIMD-only work (no
                    # PE time), and early emission frees the psY banks sooner
                    # for the next block's accumulation
                    emit(0)
                    emit(1)

                def queue_outproj(c):
                    # single-matmul granularity: each pending unit costs the
                    # PE at most 216ns, under the ~280ns/kt slack the
                    # ACT-paced attention loop leaves, so pops never slip the
                    # exp cadence (copies ride DVE, DMAs ride SP: PE-free)
                    for st in range(4):
                        qt = c * 4 + st
                        carrier = {}

                        def emit_mm(j, m, st=st, c=c, carrier=carrier):
                            if j == 0 and m == 0:
                                carrier["out_sb"] = opool.tile([P, o], F32, tag="out_sb", name="out_sb")
                            if m == 0:
                                carrier["ps_o"] = ps_opool.tile([P, CH], F32, tag="ps_o", name="ps_o")
                            nc.tensor.matmul(
                                carrier["ps_o"][:],
                                YT[m][c][:, st * P : (st + 1) * P],
                                wo_sb[:, m, j * CH : (j + 1) * CH],
                                start=(m == 0),
                                stop=(m == hd // P - 1),
                            )

                        def emit_copy(j, carrier=carrier):
                            nc.vector.tensor_copy(
                                carrier["out_sb"][:, j * CH : (j + 1) * CH],
                                carrier["ps_o"][:],
                            )

                        def emit_dma(qt=qt, carrier=carrier):
                            nc.sync.dma_start(
                                out.ap()[qt * P : (qt + 1) * P, :], carrier["out_sb"][:]
                            )

                        for j in range(2):
                            for m in range(hd // P):
                                pending.append(lambda j=j, m=m, f=emit_mm: f(j, m))
                            pending.append(lambda j=j, f=emit_copy: f(j))
                        pending.append(emit_dma)

                # block-level pipeline: during block bi's y-phase, block
                # bi+2's scores/exp stream on ACT (blocks 0/1 ran under the
                # projections), so y-matmuls run with a 2-block exp cushion
                with (
                    tc.tile_pool(name="ps_y0", bufs=2, space="PSUM") as ps_y0pool,
                    tc.tile_pool(name="ps_y1", bufs=1, space="PSUM") as ps_y1pool,
                    tc.tile_pool(name="ps_o", bufs=1, space="PSUM") as ps_opool,
                ):
                    for bi, (c, p_) in enumerate(blocks):
                        hA, hB = 2 * p_, 2 * p_ + 1
                        psY0 = ps_y0pool.tile([DH + 1, CH], F32, tag="psY0", name="psY0")
                        if bi == len(blocks) - 1:
                            # the last block's head-B accumulator borrows the
                            # (now drained) outproj bank: no wait on
                            # normalize(bi-1, h1) reading the single psY1
                            psY1f = ps_opool.tile([P, CH], F32, tag="ps_o", name="psY1f")
                            psY = [psY0, psY1f[0 : DH + 1]]
                        else:
                            psY = [
                                psY0,
                                ps_y1pool.tile([DH + 1, CH], F32, tag="psY1", name="psY1"),
                            ]
                        for kt2 in range(0, nkt, 2):
                            # y for two k-tiles first (frees both pt slots),
                            # then both score pairs back-to-back: the second
                            # pair skips the PE tile-config switch penalty
                            for kt in (kt2, kt2 + 1):
                                pt = PT.pop((bi, kt))
                                nc.tensor.matmul(
                                    psY[0][:],
                                    V[kt][:, hA, :],
                                    pt[:, 0:CH],
                                    start=(kt == 0),
                                    stop=(kt == nkt - 1),
                                )
                                nc.tensor.matmul(
                                    psY[1][:],
                                    V[kt][:, hB, :],
                                    pt[:, CH : 2 * CH],
                                    start=(kt == 0),
                                    stop=(kt == nkt - 1),
                                )
                            for kt in (kt2, kt2 + 1):
                                if bi + 2 < len(blocks) and (bi + 2, kt) not in queued:
                                    enqueue_scores(bi + 2, kt)
                                if score_q:
                                    emit_score_exp(*pop_scores())
                            npop = 4 if (bi >= len(blocks) - 2 or len(pending) > 16) else 2
                            for _ in range(npop):
                                if pending:
                                    pending.pop(0)()
                        queue_normalize(p_, c, psY, last=(bi == len(blocks) - 1))
                        if p_ == npair - 1 and c < nch - 1:
                            queue_outproj(c)
                    while pending:
                        pending.pop(0)()

                # last chunk's output projection runs after the psY pools
                # close, with a 4-deep PSUM rotation and copies alternating
                # between DVE and the now-idle ACT engine, so the drain isn't
                # serialized on a single ps_o bank
                with tc.tile_pool(name="ps_od", bufs=4, space="PSUM") as ps_odpool:
                    c = nch - 1
                    for st in range(4):
                        out_sb = opool.tile([P, o], F32, tag="out_sb", name="out_sb")
                        for j in range(2):
                            ps_o = ps_odpool.tile([P, CH], F32, tag="ps_od", name="ps_od")
                            for m in range(hd // P):
                                nc.tensor.matmul(
                                    ps_o[:],
                                    YT[m][c][:, st * P : (st + 1) * P],
                                    wo_sb[:, m, j * CH : (j + 1) * CH],
                                    start=(m == 0),
                                    stop=(m == hd // P - 1),
                                )
                            if (2 * st + j) % 2 == 0:
                                nc.vector.tensor_copy(
                                    out_sb[:, j * CH : (j + 1) * CH], ps_o[:]
                                )
                            else:
                                nc.scalar.copy(
                                    out_sb[:, j * CH : (j + 1) * CH], ps_o[:]
                                )
                        qt = c * 4 + st
                        nc.sync.dma_start(
                            out.ap()[qt * P : (qt + 1) * P, :], out_sb[:]
                        )

    nc.compile()
    return nc


def _pack_x(xT):
    """[F, S] -> [128, nch, nf, CH] with [p, c, t, n] = xT[t*128+p, c*CH+n],
    so each (partition, chunk) is one contiguous multi-KB DMA run."""
    nf, nch = F // P, S // CH
    return np.ascontiguousarray(
        xT.reshape(nf, P, nch, CH).transpose(1, 2, 0, 3)
    ).astype(np.float16)


def _pack_w(w2d):
    """[F, hd] -> [128, nf, hd] with [p, t, :] = w2d[t*128+p, :]."""
    nf = F // P
    hd = w2d.shape[1]
    return np.ascontiguousarray(w2d.reshape(nf, P, hd).transpose(1, 0, 2)).astype(
        np.float16
    )


def make_in_maps(inputs_q, inputs_kv, wq, wk, wv, wo):
    """Shard full inputs into 8 per-core input dicts (host-side)."""
    in_maps = []
    scale = 1.0 / np.sqrt(DH)
    hd = HPC * DH
    for core in range(NCORES):
        b = core // (NCORES // B)
        hg = core % (NCORES // B)
        hs = slice(hg * HPC, (hg + 1) * HPC)
        in_maps.append(
            {
                "xqT": _pack_x(inputs_q[b].T),
                "xkvT": _pack_x(inputs_kv[b].T),
                "wq": _pack_w((wq[:, hs, :] * scale).reshape(F, hd)),
                "wk": _pack_w(wk[:, hs, :].reshape(F, hd)),
                "wv": _pack_w(wv[:, hs, :].reshape(F, hd)),
                "wo": np.ascontiguousarray(
                    wo[hs].reshape(hd, O).reshape(hd // P, P, O).transpose(1, 0, 2)
                ).astype(np.float16),
            }
        )
    return in_maps


_CACHE = {}


def _get_program():
    if "nc" not in _CACHE:
        _CACHE["nc"] = build_program()
    return _CACHE["nc"]


def run_sharded(inputs_q, inputs_kv, wq, wk, wv, wo, bo, **spmd_kwargs):
    """Build in_maps, run on 8 cores, reduce partials. Returns (out, results)."""
    nc = _get_program()
    in_maps = make_in_maps(inputs_q, inputs_kv, wq, wk, wv, wo)
    res = run_bass_kernel_spmd(nc, in_maps, core_ids=list(range(NCORES)), **spmd_kwargs)
    gpb = NCORES // B  # head-group cores per batch element
    out = np.zeros((B, S, O), dtype=np.float32)
    for core in range(NCORES):
        out[core // gpb] += res.results[core]["out"]
    out += np.asarray(bo, dtype=np.float32)
    return out, res


def kernel(inputs_q, inputs_kv, wq, wk, wv, wo, bo):
    out, _ = run_sharded(
        np.asarray(inputs_q),
        np.asarray(inputs_kv),
        np.asarray(wq),
        np.asarray(wk),
        np.asarray(wv),
        np.asarray(wo),
        np.asarray(bo),
    )
    return out
